# revision 13
# baseline (speedup 1.0000x reference)
"""Cross-frame attention kernel for 8 TRN2 NeuronCores.

Sharding: core c handles batch b = c//2 and head-group g = c%2 (4 of the 8
heads).  The host pre-transposes x[b]/context[b] (feature dim onto SBUF
partitions) and casts to bf16; each core computes a partial output
(its 4 heads pushed through the matching Wo rows) and the host sums the
two partials per batch plus the bias.

Device math per core (S^T layout, softmax over the partition j-dim):
  QT = Wq_g^T x^T          [256, 2048]
  KT = Wk_g^T c^T          [256, 2048]
  V  = c Wv_g              [2048, 256] (+ ones column per head)
  S^T = K_h Q_h^T          [j, i] tiles, exp via ScalarE (scale=1/8 fused)
  O~^T | Z = [V_h|1]^T expS^T   (PSUM accumulate over j)
  A^T = O~^T * bcast(1/Z)  (K=1 broadcast matmul for the free-dim scale)
  out_partial = A^T^T Wo_g [2048, 512] fp32

Schedule: 16 attention blocks (m-major).  Within a block the PV matmuls of
j-group g-1 issue after S/exp of group g, so the PE never waits on the
just-issued exp.  Each block's PV tail + softmax normalization carry into
the next block's early groups, and projection / output-projection chunks
drip one per group slot to keep the PE busy through Act-bound stretches.
PSUM->SBUF copies run on the Pool engine (via gpsimd), normalization
element-wise ops on DVE, exp exclusively on the Act engine.

Logits are |S/8| <~ 1.1 for this problem's scale, so softmax without
max-subtraction is exact in fp32.
"""

import numpy as np
import ml_dtypes

B = 4
N = 2048  # query length
M = 2048  # context length
DIM = 512
HEADS = 8
DH = 64
HC = 256  # head columns handled per core (4 heads)
P = 128
KO = DIM // P  # 4 k-chunks
NI4 = N // 512  # 4 i-chunks of 512
NJ = M // P  # 16 j-chunks
JPG = 2  # j-chunks per exp group (PSUM banks per S^T buffer)
NG = NJ // JPG  # 8 groups per block

_CACHE = {}


def _build():
    from contextlib import ExitStack

    import concourse.mybir as mybir
    import concourse.tile as tile
    from concourse import bacc

    bf = mybir.dt.bfloat16
    f32 = mybir.dt.float32
    Exp = mybir.ActivationFunctionType.Exp

    nc = bacc.Bacc(None, target_bir_lowering=False, debug=False)
    with tile.TileContext(nc) as tc:
        with ExitStack() as ctx:
            dram = ctx.enter_context(tc.tile_pool(name="dram", bufs=1, space="DRAM"))
            xT_d = dram.tile([DIM, N], bf, kind="ExternalInput")
            cT_d = dram.tile([DIM, M], bf, kind="ExternalInput")
            wq_d = dram.tile([DIM, HC], bf, kind="ExternalInput")
            wk_d = dram.tile([DIM, HC], bf, kind="ExternalInput")
            wv_d = dram.tile([DIM, HC], bf, kind="ExternalInput")
            wo_d = dram.tile([HC, DIM], bf, kind="ExternalInput")
            out_d = dram.tile([N, DIM], f32, kind="ExternalOutput")

            const = ctx.enter_context(tc.tile_pool(name="const", bufs=1))

            xt_sb = const.tile([P, KO, N], bf, tag="xt")
            ct_sb = const.tile([P, KO, M], bf, tag="ct")
            wq_sb = const.tile([P, KO, HC], bf, tag="wq")
            wk_sb = const.tile([P, KO, HC], bf, tag="wk")
            wv_sb = const.tile([P, KO, HC], bf, tag="wv")
            wo_sb = const.tile([P, 2, DIM], bf, tag="wo")
            qT_sb = const.tile([P, 2, N], bf, tag="qT")
            kT_sb = const.tile([P, 2, M], bf, tag="kT")
            # all 4 heads' V with a trailing ones column: [j, jo, head, 65]
            vp_sb = const.tile([P, NJ, 4, DH + 1], bf, tag="vp")
            aT_sb = const.tile([P, 2, N], bf, tag="aT")
            ones_sb = const.tile([1, DH], bf, tag="ones1")

            dummy_sb = const.tile([1, 1], f32, tag="dummy")
            nc.vector.memset(ones_sb[:], 1.0)
            nc.vector.memset(vp_sb[:, :, :, DH : DH + 1], 1.0)
            # hoist the exp ACT-table load out of the critical path
            nc.scalar.activation(dummy_sb[:], ones_sb[0:1, 0:1], Exp, scale=1.0)

            # DMA in, first-needed first and merged into few transfers:
            # the HWDGE dispatcher costs ~625ns per DMA, so per-ko pieces
            # only for the very first chunk (earliest matmul start), whole
            # tiles for everything else.
            cT_r = cT_d[:].rearrange("(ko p) i -> p ko i", p=P)
            xT_r = xT_d[:].rearrange("(ko p) i -> p ko i", p=P)
            wk_r = wk_d[:].rearrange("(ko p) m -> p ko m", p=P)
            wq_r = wq_d[:].rearrange("(ko p) m -> p ko m", p=P)
            nc.sync.dma_start(wk_sb[:, 0, :], wk_r[:, 0, :])
            nc.sync.dma_start(ct_sb[:, 0, 0:512], cT_r[:, 0, 0:512])
            nc.sync.dma_start(wq_sb[:, 0, :], wq_r[:, 0, :])
            nc.sync.dma_start(xt_sb[:, 0, 0:512], xT_r[:, 0, 0:512])
            nc.sync.dma_start(wk_sb[:, 1:, :], wk_r[:, 1:, :])
            nc.sync.dma_start(ct_sb[:, 1:, 0:512], cT_r[:, 1:, 0:512])
            nc.sync.dma_start(wq_sb[:, 1:, :], wq_r[:, 1:, :])
            nc.sync.dma_start(xt_sb[:, 1:, 0:512], xT_r[:, 1:, 0:512])
            nc.sync.dma_start(wv_sb[:], wv_d[:].rearrange("(ko p) m -> p ko m", p=P))
            for i4 in range(1, NI4):
                isl = slice(i4 * 512, (i4 + 1) * 512)
                nc.sync.dma_start(ct_sb[:, :, isl], cT_r[:, :, isl])
                nc.sync.dma_start(xt_sb[:, :, isl], xT_r[:, :, isl])
            nc.sync.dma_start(wo_sb[:], wo_d[:].rearrange("(r p) n -> p r n", p=P))

            # Single shared PSUM budget (8 banks):
            #   s-tag 2x2 + o 2 + aux 1 + scr 1
            with (
                tc.tile_pool(name="s_ps", bufs=2, space="PSUM") as s_pool,
                tc.tile_pool(name="aux_ps", bufs=1, space="PSUM") as aux_pool,
                tc.tile_pool(name="o_ps", bufs=2, space="PSUM") as o_pool,
                tc.tile_pool(name="scr_ps", bufs=1, space="PSUM") as scr_pool,
                tc.tile_pool(name="e_sb", bufs=4) as e_pool,
                tc.tile_pool(name="small", bufs=2) as small,
                tc.tile_pool(name="ost", bufs=6) as ostp,
            ):
                Copy = mybir.ActivationFunctionType.Copy

                def qk_proj(wsb, src_sb, dst, m, c, pool=None, eng="dve"):
                    pool = pool or aux_pool
                    tg = "aux" if pool is aux_pool else "scr"
                    ps = pool.tile([P, 512], f32, tag=tg, name="ps_qk")
                    for ko in range(KO):
                        nc.tensor.matmul(
                            ps[:],
                            wsb[:, ko, m * P : (m + 1) * P],
                            src_sb[:, ko, c * 512 : (c + 1) * 512],
                            start=(ko == 0),
                            stop=(ko == KO - 1),
                        )
                    dsl = dst[:, m, c * 512 : (c + 1) * 512]
                    if eng == "act":
                        nc.scalar.activation(dsl, ps[:], Copy)
                    else:
                        nc.vector.tensor_copy(dsl, ps[:])

                def kchunk(m, c, pool=None, eng="dve"):
                    return lambda: qk_proj(wk_sb, ct_sb, kT_sb, m, c, pool, eng)

                def qchunk(m, c, pool=None, eng="dve"):
                    return lambda: qk_proj(wq_sb, xt_sb, qT_sb, m, c, pool, eng)

                def vpair(g, pools=None):
                    def f():
                        for idx, jo in enumerate((2 * g, 2 * g + 1)):
                            pool = (pools or (aux_pool, aux_pool))[idx]
                            tg = "aux" if pool is aux_pool else "scr"
                            ps = pool.tile([P, HC], f32, tag=tg, name="ps_v")
                            for ko in range(KO):
                                nc.tensor.matmul(
                                    ps[:],
                                    ct_sb[:, ko, jo * P : (jo + 1) * P],
                                    wv_sb[:, ko, :],
                                    start=(ko == 0),
                                    stop=(ko == KO - 1),
                                )
                            nc.vector.tensor_copy(
                                vp_sb[:, jo, :, 0:DH],
                                ps[:].rearrange("p (h d) -> p h d", h=4),
                            )
                    return f

                ost_tiles = {}

                def wo_chunk(i, pool=None, tail=False):
                    def f():
                        pool_ = pool or scr_pool
                        tg = "aux" if pool_ is aux_pool else "scr"
                        ps = pool_.tile([P, DIM], f32, tag=tg, name="p3_ps")
                        for m in range(2):
                            nc.tensor.matmul(
                                ps[:],
                                aT_sb[:, m, i * P : (i + 1) * P],
                                wo_sb[:, m, :],
                                start=(m == 0),
                                stop=(m == 1),
                            )
                        if tail:
                            # latency mode: Act-engine copy (DVE is busy with
                            # the apply pieces), single unpaired DMA
                            ost = ostp.tile([P, DIM], f32, tag="ost1", name="ost")
                            nc.scalar.activation(ost[:], ps[:], Copy)
                            nc.sync.dma_start(out_d[i * P : (i + 1) * P, :], ost[:])
                            return
                        # chunk pairs share one ost tile and one output DMA
                        # (halves the ~625ns/DMA HWDGE dispatch cost)
                        pi, half = divmod(i, 2)
                        if half == 0:
                            ost_tiles[pi] = ostp.tile(
                                [P, 2, DIM], f32, tag="ost", name="ost"
                            )
                        ost = ost_tiles[pi]
                        nc.vector.tensor_copy(ost[:, half, :], ps[:])
                        if half == 1:
                            nc.sync.dma_start(
                                out_d[pi * 256 : (pi + 1) * 256, :].rearrange(
                                    "(h p) n -> p h n", p=P
                                ),
                                ost[:],
                            )
                    return f

                def att_block(i4, m, hl, carry=None, fillers=None, last=False):
                    """One (i4, m, hl) attention block.  Returns the carry
                    closures [pv_tail, recip, bc, apply] that the NEXT block
                    (or the tail) must run in order at its first 4 groups."""
                    isl = slice(i4 * 512, (i4 + 1) * 512)
                    h = 2 * m + hl
                    pb = DH * hl
                    o_ps = o_pool.tile([DH + 1, 512], f32, tag="o", name="o_ps")
                    e_tiles = [None] * NG

                    def s_and_exp(g):
                        s_ps = s_pool.tile([P, JPG, 512], f32, tag="s", name="s_ps")
                        for jj in range(JPG):
                            j = g * JPG + jj
                            nc.tensor.matmul(
                                s_ps[:, jj, :],
                                kT_sb[pb : pb + DH, m, j * P : (j + 1) * P],
                                qT_sb[pb : pb + DH, m, isl],
                                start=True,
                                stop=True,
                            )
                        e_sb = e_pool.tile([P, JPG, 512], bf, tag="e", name="e_sb")
                        nc.scalar.activation(e_sb[:], s_ps[:], Exp, scale=0.125)
                        e_tiles[g] = e_sb

                    def pv(g):
                        for jj in range(JPG):
                            j = g * JPG + jj
                            nc.tensor.matmul(
                                o_ps[:],
                                vp_sb[:, j, h, :],
                                e_tiles[g][:, jj, :],
                                start=(j == 0),
                                stop=(j == NJ - 1),
                            )

                    lag = 1 if last else 2
                    for g in range(NG):
                        s_and_exp(g)
                        if g > lag - 1:
                            pv(g - lag)
                        if carry is not None and g < len(carry):
                            carry[g]()
                        for f in (fillers or {}).get(g, []):
                            f()

                    rzb = small.tile([1, 512], bf, tag="rzb", name="rzb")
                    if last:
                        bc = o_pool.tile([DH, 512], f32, tag="o", name="bc")
                    else:
                        bc = scr_pool.tile([DH, 512], f32, tag="scr", name="bc")
                    bcb = small.tile([DH, 512], bf, tag="bcb", name="bcb")

                    def recip():
                        # bf16 out is as precise as the old f32->bf16 copy
                        with nc.allow_low_precision(reason="1/Z used in bf16"):
                            nc.vector.reciprocal(rzb[:], o_ps[DH : DH + 1, :])

                    def bcf():
                        nc.tensor.matmul(
                            bc[:], ones_sb[:], rzb[:], start=True, stop=True
                        )

                    def apply():
                        nc.vector.tensor_copy(bcb[:], bc[:])
                        nc.vector.tensor_mul(
                            aT_sb[pb : pb + DH, m, isl], o_ps[0:DH, :], bcb[:]
                        )

                    def apply_piece(ii):
                        # 128-col slice so the tail's wo chunks can chase it;
                        # bcb copy on Act (idle after the final exp)
                        csl = slice(ii * P, (ii + 1) * P)
                        asl = slice(i4 * 512 + ii * P, i4 * 512 + (ii + 1) * P)

                        def f():
                            nc.scalar.activation(bcb[:, csl], bc[:, csl], Copy)
                            nc.vector.tensor_mul(
                                aT_sb[pb : pb + DH, m, asl],
                                o_ps[0:DH, csl],
                                bcb[:, csl],
                            )
                        return f

                    if last:
                        tail_carry = [lambda: pv(NG - 1), recip, bcf]
                    else:
                        tail_carry = [
                            lambda: pv(NG - 2),
                            lambda: pv(NG - 1),
                            recip,
                            bcf,
                            apply,
                        ]
                    return tail_carry, apply_piece

                # --- schedule: m-major over 16 blocks -------------------
                qk_proj(wk_sb, ct_sb, kT_sb, 0, 0, aux_pool, "act")
                qk_proj(wq_sb, xt_sb, qT_sb, 0, 0, scr_pool, "dve")

                order = [
                    (i4, m, hl) for m in (0, 1) for i4 in range(4) for hl in (0, 1)
                ]
                fillers = {
                    0: {  # (0,0,0): kT0 chunks at fixed slots + all V pairs
                        0: [vpair(0)],
                        1: [kchunk(0, 1, scr_pool, "act"),
                            vpair(1, (aux_pool, scr_pool))],
                        2: [vpair(2)],
                        3: [kchunk(0, 2, scr_pool, "act"), vpair(3)],
                        4: [vpair(4, (aux_pool, scr_pool))],
                        5: [kchunk(0, 3, scr_pool, "act"),
                            vpair(5, (aux_pool, scr_pool))],
                        6: [vpair(6, (aux_pool, scr_pool))],
                        7: [vpair(7, (aux_pool, scr_pool))],
                    },
                    1: {5: [qchunk(0, 1)], 7: [qchunk(0, 2)]},  # (0,0,1)
                    2: {4: [qchunk(0, 3)]},       # (1,0,0)
                    3: {4: [kchunk(1, 0)]},       # (1,0,1)
                    4: {4: [kchunk(1, 1)]},       # (2,0,0)
                    5: {4: [kchunk(1, 2)]},       # (2,0,1)
                    6: {4: [kchunk(1, 3)], 6: [qchunk(1, 0)]},  # (3,0,0)
                    7: {4: [qchunk(1, 1)]},       # (3,0,1)
                    8: {4: [qchunk(1, 2)]},       # (0,1,0)
                    9: {},                        # (0,1,1)
                    10: {5: [wo_chunk(0)], 7: [wo_chunk(1, aux_pool)]},  # (1,1,0)
                    11: {4: [qchunk(1, 3)], 5: [wo_chunk(2)],
                         7: [wo_chunk(3, aux_pool)]},           # (1,1,1)
                    12: {5: [wo_chunk(4)], 7: [wo_chunk(5, aux_pool)]},  # (2,1,0)
                    13: {5: [wo_chunk(6)], 7: [wo_chunk(7, aux_pool)]},  # (2,1,1)
                    14: {5: [wo_chunk(8)], 7: [wo_chunk(9, aux_pool)]},  # (3,1,0)
                    15: {5: [wo_chunk(10)], 7: [wo_chunk(11, aux_pool)]},  # (3,1,1)
                }
                carry = None
                for bi, (i4, m, hl) in enumerate(order):
                    carry, apply_piece = att_block(
                        i4, m, hl, carry, fillers.get(bi), last=(bi == 15)
                    )
                # tail: finish the last block with the apply split into
                # 128-col pieces, each chased by its wo chunk
                for f in carry[:3]:
                    f()
                for ii in range(4):
                    apply_piece(ii)()
                    wo_chunk(12 + ii, (scr_pool, aux_pool)[ii % 2], tail=True)()

    nc.compile()
    names = dict(
        xT=xT_d.name,
        cT=cT_d.name,
        wq=wq_d.name,
        wk=wk_d.name,
        wv=wv_d.name,
        wo=wo_d.name,
        out=out_d.name,
    )
    return nc, names


def _get_built():
    if "nc" not in _CACHE:
        _CACHE["nc"], _CACHE["names"] = _build()
    return _CACHE["nc"], _CACHE["names"]


def run(x, context, Wq, Wk, Wv, Wo, bo, trace=False):
    from concourse.bass_utils import run_bass_kernel_spmd

    nc, names = _get_built()
    bf16 = ml_dtypes.bfloat16

    x = np.asarray(x, dtype=np.float32)
    context = np.asarray(context, dtype=np.float32)
    Wq = np.asarray(Wq, dtype=np.float32)
    Wk = np.asarray(Wk, dtype=np.float32)
    Wv = np.asarray(Wv, dtype=np.float32)
    Wo = np.asarray(Wo, dtype=np.float32)
    bo = np.asarray(bo, dtype=np.float32)

    in_maps = []
    for c in range(8):
        b, g = divmod(c, 2)
        cols = slice(g * HC, (g + 1) * HC)
        in_maps.append(
            {
                names["xT"]: np.ascontiguousarray(x[b].T).astype(bf16),
                names["cT"]: np.ascontiguousarray(context[b].T).astype(bf16),
                names["wq"]: np.ascontiguousarray(Wq[:, cols]).astype(bf16),
                names["wk"]: np.ascontiguousarray(Wk[:, cols]).astype(bf16),
                names["wv"]: np.ascontiguousarray(Wv[:, cols]).astype(bf16),
                names["wo"]: np.ascontiguousarray(Wo[cols, :]).astype(bf16),
            }
        )

    res = run_bass_kernel_spmd(
        nc, in_maps, core_ids=list(range(8)), trace=trace,
        stitch_traces=trace,
    )
    out = np.empty((B, N, DIM), dtype=np.float32)
    for b in range(B):
        out[b] = res.results[2 * b][names["out"]] + res.results[2 * b + 1][names["out"]]
    out += bo[None, None, :]
    return out, res


def kernel(x, context, Wq, Wk, Wv, Wo, bo):
    out, _ = run(x, context, Wq, Wk, Wv, Wo, bo, trace=False)
    return out


# revision 14
# speedup vs baseline: 1.0039x; 1.0039x over previous
"""Cross-frame attention kernel for 8 TRN2 NeuronCores.

Sharding: core c handles batch b = c//2 and head-group g = c%2 (4 of the 8
heads).  The host pre-transposes x[b]/context[b] (feature dim onto SBUF
partitions) and casts to bf16; each core computes a partial output
(its 4 heads pushed through the matching Wo rows) and the host sums the
two partials per batch plus the bias.

Device math per core (S^T layout, softmax over the partition j-dim):
  QT = Wq_g^T x^T          [256, 2048]
  KT = Wk_g^T c^T          [256, 2048]
  V  = c Wv_g              [2048, 256] (+ ones column per head)
  S^T = K_h Q_h^T          [j, i] tiles, exp via ScalarE (scale=1/8 fused)
  O~^T | Z = [V_h|1]^T expS^T   (PSUM accumulate over j)
  A^T = O~^T * bcast(1/Z)  (K=1 broadcast matmul for the free-dim scale)
  out_partial = A^T^T Wo_g [2048, 512] fp32

Schedule: 16 attention blocks (m-major).  Within a block the PV matmuls of
j-group g-1 issue after S/exp of group g, so the PE never waits on the
just-issued exp.  Each block's PV tail + softmax normalization carry into
the next block's early groups, and projection / output-projection chunks
drip one per group slot to keep the PE busy through Act-bound stretches.
PSUM->SBUF copies run on the Pool engine (via gpsimd), normalization
element-wise ops on DVE, exp exclusively on the Act engine.

Logits are |S/8| <~ 1.1 for this problem's scale, so softmax without
max-subtraction is exact in fp32.
"""

import numpy as np
import ml_dtypes

B = 4
N = 2048  # query length
M = 2048  # context length
DIM = 512
HEADS = 8
DH = 64
HC = 256  # head columns handled per core (4 heads)
P = 128
KO = DIM // P  # 4 k-chunks
NI4 = N // 512  # 4 i-chunks of 512
NJ = M // P  # 16 j-chunks
JPG = 2  # j-chunks per exp group (PSUM banks per S^T buffer)
NG = NJ // JPG  # 8 groups per block

_CACHE = {}


def _build():
    from contextlib import ExitStack

    import concourse.mybir as mybir
    import concourse.tile as tile
    from concourse import bacc

    bf = mybir.dt.bfloat16
    f32 = mybir.dt.float32
    Exp = mybir.ActivationFunctionType.Exp

    nc = bacc.Bacc(None, target_bir_lowering=False, debug=False)
    with tile.TileContext(nc) as tc:
        with ExitStack() as ctx:
            dram = ctx.enter_context(tc.tile_pool(name="dram", bufs=1, space="DRAM"))
            xT_d = dram.tile([DIM, N], bf, kind="ExternalInput")
            cT_d = dram.tile([DIM, M], bf, kind="ExternalInput")
            wq_d = dram.tile([DIM, HC], bf, kind="ExternalInput")
            wk_d = dram.tile([DIM, HC], bf, kind="ExternalInput")
            wv_d = dram.tile([DIM, HC], bf, kind="ExternalInput")
            wo_d = dram.tile([HC, DIM], bf, kind="ExternalInput")
            out_d = dram.tile([N, DIM], f32, kind="ExternalOutput")

            const = ctx.enter_context(tc.tile_pool(name="const", bufs=1))

            xt_sb = const.tile([P, KO, N], bf, tag="xt")
            ct_sb = const.tile([P, KO, M], bf, tag="ct")
            wq_sb = const.tile([P, KO, HC], bf, tag="wq")
            wk_sb = const.tile([P, KO, HC], bf, tag="wk")
            wv_sb = const.tile([P, KO, HC], bf, tag="wv")
            wo_sb = const.tile([P, 2, DIM], bf, tag="wo")
            qT_sb = const.tile([P, 2, N], bf, tag="qT")
            kT_sb = const.tile([P, 2, M], bf, tag="kT")
            # all 4 heads' V with a trailing ones column: [j, jo, head, 65]
            vp_sb = const.tile([P, NJ, 4, DH + 1], bf, tag="vp")
            aT_sb = const.tile([P, 2, N], bf, tag="aT")
            ones_sb = const.tile([1, DH], bf, tag="ones1")

            dummy_sb = const.tile([1, 1], f32, tag="dummy")
            nc.vector.memset(ones_sb[:], 1.0)
            nc.vector.memset(vp_sb[:, :, :, DH : DH + 1], 1.0)
            # hoist the exp ACT-table load out of the critical path
            nc.scalar.activation(dummy_sb[:], ones_sb[0:1, 0:1], Exp, scale=1.0)

            # DMA in, first-needed first and merged into few transfers:
            # the HWDGE dispatcher costs ~625ns per DMA, so per-ko pieces
            # only for the very first chunk (earliest matmul start), whole
            # tiles for everything else.
            cT_r = cT_d[:].rearrange("(ko p) i -> p ko i", p=P)
            xT_r = xT_d[:].rearrange("(ko p) i -> p ko i", p=P)
            wk_r = wk_d[:].rearrange("(ko p) m -> p ko m", p=P)
            wq_r = wq_d[:].rearrange("(ko p) m -> p ko m", p=P)
            nc.sync.dma_start(wk_sb[:, 0, :], wk_r[:, 0, :])
            nc.sync.dma_start(ct_sb[:, 0, 0:512], cT_r[:, 0, 0:512])
            nc.sync.dma_start(wq_sb[:, 0, :], wq_r[:, 0, :])
            nc.sync.dma_start(xt_sb[:, 0, 0:512], xT_r[:, 0, 0:512])
            nc.sync.dma_start(wk_sb[:, 1:, :], wk_r[:, 1:, :])
            nc.sync.dma_start(ct_sb[:, 1:, 0:512], cT_r[:, 1:, 0:512])
            nc.sync.dma_start(wq_sb[:, 1:, :], wq_r[:, 1:, :])
            nc.sync.dma_start(xt_sb[:, 1:, 0:512], xT_r[:, 1:, 0:512])
            nc.sync.dma_start(wv_sb[:], wv_d[:].rearrange("(ko p) m -> p ko m", p=P))
            for i4 in range(1, NI4):
                isl = slice(i4 * 512, (i4 + 1) * 512)
                nc.sync.dma_start(ct_sb[:, :, isl], cT_r[:, :, isl])
                nc.sync.dma_start(xt_sb[:, :, isl], xT_r[:, :, isl])
            nc.sync.dma_start(wo_sb[:], wo_d[:].rearrange("(r p) n -> p r n", p=P))

            # Single shared PSUM budget (8 banks):
            #   s-tag 2x2 + o 2 + aux 1 + scr 1
            with (
                tc.tile_pool(name="s_ps", bufs=2, space="PSUM") as s_pool,
                tc.tile_pool(name="aux_ps", bufs=1, space="PSUM") as aux_pool,
                tc.tile_pool(name="o_ps", bufs=2, space="PSUM") as o_pool,
                tc.tile_pool(name="scr_ps", bufs=1, space="PSUM") as scr_pool,
                tc.tile_pool(name="e_sb", bufs=4) as e_pool,
                tc.tile_pool(name="small", bufs=2) as small,
                tc.tile_pool(name="ost", bufs=6) as ostp,
            ):
                Copy = mybir.ActivationFunctionType.Copy

                def qk_proj(wsb, src_sb, dst, m, c, pool=None, eng="dve"):
                    pool = pool or aux_pool
                    tg = "aux" if pool is aux_pool else "scr"
                    ps = pool.tile([P, 512], f32, tag=tg, name="ps_qk")
                    for ko in range(KO):
                        nc.tensor.matmul(
                            ps[:],
                            wsb[:, ko, m * P : (m + 1) * P],
                            src_sb[:, ko, c * 512 : (c + 1) * 512],
                            start=(ko == 0),
                            stop=(ko == KO - 1),
                        )
                    dsl = dst[:, m, c * 512 : (c + 1) * 512]
                    if eng == "act":
                        nc.scalar.activation(dsl, ps[:], Copy)
                    else:
                        nc.vector.tensor_copy(dsl, ps[:])

                def kchunk(m, c, pool=None, eng="dve"):
                    return lambda: qk_proj(wk_sb, ct_sb, kT_sb, m, c, pool, eng)

                def qchunk(m, c, pool=None, eng="dve"):
                    return lambda: qk_proj(wq_sb, xt_sb, qT_sb, m, c, pool, eng)

                def vpair(g, pools=None):
                    def f():
                        for idx, jo in enumerate((2 * g, 2 * g + 1)):
                            pool = (pools or (aux_pool, aux_pool))[idx]
                            tg = "aux" if pool is aux_pool else "scr"
                            ps = pool.tile([P, HC], f32, tag=tg, name="ps_v")
                            for ko in range(KO):
                                nc.tensor.matmul(
                                    ps[:],
                                    ct_sb[:, ko, jo * P : (jo + 1) * P],
                                    wv_sb[:, ko, :],
                                    start=(ko == 0),
                                    stop=(ko == KO - 1),
                                )
                            nc.vector.tensor_copy(
                                vp_sb[:, jo, :, 0:DH],
                                ps[:].rearrange("p (h d) -> p h d", h=4),
                            )
                    return f

                ost_tiles = {}

                def wo_chunk(i, pool=None, tail=False):
                    def f():
                        pool_ = pool or scr_pool
                        tg = "aux" if pool_ is aux_pool else "scr"
                        ps = pool_.tile([P, DIM], f32, tag=tg, name="p3_ps")
                        for m in range(2):
                            nc.tensor.matmul(
                                ps[:],
                                aT_sb[:, m, i * P : (i + 1) * P],
                                wo_sb[:, m, :],
                                start=(m == 0),
                                stop=(m == 1),
                            )
                        if tail:
                            # latency mode: Act-engine copy (DVE is busy with
                            # the apply pieces), single unpaired DMA
                            ost = ostp.tile([P, DIM], f32, tag="ost1", name="ost")
                            nc.scalar.activation(ost[:], ps[:], Copy)
                            nc.sync.dma_start(out_d[i * P : (i + 1) * P, :], ost[:])
                            return
                        # chunk pairs share one ost tile and one output DMA
                        # (halves the ~625ns/DMA HWDGE dispatch cost)
                        pi, half = divmod(i, 2)
                        if half == 0:
                            ost_tiles[pi] = ostp.tile(
                                [P, 2, DIM], f32, tag="ost", name="ost"
                            )
                        ost = ost_tiles[pi]
                        nc.vector.tensor_copy(ost[:, half, :], ps[:])
                        if half == 1:
                            nc.sync.dma_start(
                                out_d[pi * 256 : (pi + 1) * 256, :].rearrange(
                                    "(h p) n -> p h n", p=P
                                ),
                                ost[:],
                            )
                    return f

                def att_block(i4, m, hl, carry=None, fillers=None, last=False):
                    """One (i4, m, hl) attention block.  Returns the carry
                    closures [pv_tail, recip, bc, apply] that the NEXT block
                    (or the tail) must run in order at its first 4 groups."""
                    isl = slice(i4 * 512, (i4 + 1) * 512)
                    h = 2 * m + hl
                    pb = DH * hl
                    o_ps = o_pool.tile([DH + 1, 512], f32, tag="o", name="o_ps")
                    e_tiles = [None] * NG

                    def s_and_exp(g):
                        s_ps = s_pool.tile([P, JPG, 512], f32, tag="s", name="s_ps")
                        for jj in range(JPG):
                            j = g * JPG + jj
                            nc.tensor.matmul(
                                s_ps[:, jj, :],
                                kT_sb[pb : pb + DH, m, j * P : (j + 1) * P],
                                qT_sb[pb : pb + DH, m, isl],
                                start=True,
                                stop=True,
                            )
                        e_sb = e_pool.tile([P, JPG, 512], bf, tag="e", name="e_sb")
                        nc.scalar.activation(e_sb[:], s_ps[:], Exp, scale=0.125)
                        e_tiles[g] = e_sb

                    def pv(g):
                        for jj in range(JPG):
                            j = g * JPG + jj
                            nc.tensor.matmul(
                                o_ps[:],
                                vp_sb[:, j, h, :],
                                e_tiles[g][:, jj, :],
                                start=(j == 0),
                                stop=(j == NJ - 1),
                            )

                    for g in range(NG):
                        s_and_exp(g)
                        if g > 1:
                            pv(g - 2)
                        if carry is not None and g < len(carry):
                            carry[g]()
                        for f in (fillers or {}).get(g, []):
                            f()

                    rzb = small.tile([1, 512], bf, tag="rzb", name="rzb")
                    if last:
                        bc = o_pool.tile([DH, 512], f32, tag="o", name="bc")
                    else:
                        bc = scr_pool.tile([DH, 512], f32, tag="scr", name="bc")
                    bcb = small.tile([DH, 512], bf, tag="bcb", name="bcb")

                    def recip():
                        # bf16 out is as precise as the old f32->bf16 copy
                        with nc.allow_low_precision(reason="1/Z used in bf16"):
                            nc.vector.reciprocal(rzb[:], o_ps[DH : DH + 1, :])

                    def bcf():
                        nc.tensor.matmul(
                            bc[:], ones_sb[:], rzb[:], start=True, stop=True
                        )

                    def apply():
                        nc.vector.tensor_copy(bcb[:], bc[:])
                        nc.vector.tensor_mul(
                            aT_sb[pb : pb + DH, m, isl], o_ps[0:DH, :], bcb[:]
                        )

                    def apply_piece(ii):
                        # 128-col slice so the tail's wo chunks can chase it;
                        # bcb copy on Act (idle after the final exp)
                        csl = slice(ii * P, (ii + 1) * P)
                        asl = slice(i4 * 512 + ii * P, i4 * 512 + (ii + 1) * P)

                        def f():
                            nc.scalar.activation(bcb[:, csl], bc[:, csl], Copy)
                            nc.vector.tensor_mul(
                                aT_sb[pb : pb + DH, m, asl],
                                o_ps[0:DH, csl],
                                bcb[:, csl],
                            )
                        return f

                    return [
                        lambda: pv(NG - 2),
                        lambda: pv(NG - 1),
                        recip,
                        bcf,
                        apply,
                    ], apply_piece

                # --- schedule: m-major over 16 blocks -------------------
                qk_proj(wk_sb, ct_sb, kT_sb, 0, 0, aux_pool, "act")
                qk_proj(wq_sb, xt_sb, qT_sb, 0, 0, scr_pool, "dve")

                order = [
                    (i4, m, hl) for m in (0, 1) for i4 in range(4) for hl in (0, 1)
                ]
                fillers = {
                    0: {  # (0,0,0): kT0 chunks at fixed slots + all V pairs
                        0: [vpair(0)],
                        1: [kchunk(0, 1, scr_pool, "act"),
                            vpair(1, (aux_pool, scr_pool))],
                        2: [vpair(2)],
                        3: [kchunk(0, 2, scr_pool, "act"), vpair(3)],
                        4: [vpair(4, (aux_pool, scr_pool))],
                        5: [kchunk(0, 3, scr_pool, "act"),
                            vpair(5, (aux_pool, scr_pool))],
                        6: [vpair(6, (aux_pool, scr_pool))],
                        7: [vpair(7, (aux_pool, scr_pool))],
                    },
                    1: {5: [qchunk(0, 1)], 7: [qchunk(0, 2)]},  # (0,0,1)
                    2: {4: [qchunk(0, 3)]},       # (1,0,0)
                    3: {4: [kchunk(1, 0)]},       # (1,0,1)
                    4: {4: [kchunk(1, 1)]},       # (2,0,0)
                    5: {4: [kchunk(1, 2)]},       # (2,0,1)
                    6: {4: [kchunk(1, 3)], 6: [qchunk(1, 0)]},  # (3,0,0)
                    7: {4: [qchunk(1, 1)]},       # (3,0,1)
                    8: {4: [qchunk(1, 2)]},       # (0,1,0)
                    9: {},                        # (0,1,1)
                    10: {5: [wo_chunk(0)], 7: [wo_chunk(1, aux_pool)]},  # (1,1,0)
                    11: {4: [qchunk(1, 3)], 5: [wo_chunk(2)],
                         7: [wo_chunk(3, aux_pool)]},           # (1,1,1)
                    12: {5: [wo_chunk(4)], 7: [wo_chunk(5, aux_pool)]},  # (2,1,0)
                    13: {5: [wo_chunk(6)], 7: [wo_chunk(7, aux_pool)]},  # (2,1,1)
                    14: {5: [wo_chunk(8)], 7: [wo_chunk(9, aux_pool)]},  # (3,1,0)
                    15: {5: [wo_chunk(10)], 7: [wo_chunk(11, aux_pool)]},  # (3,1,1)
                }
                carry = None
                for bi, (i4, m, hl) in enumerate(order):
                    carry, apply_piece = att_block(
                        i4, m, hl, carry, fillers.get(bi), last=(bi == 15)
                    )
                # tail: finish the last block with the apply split into
                # 128-col pieces, each chased by its wo chunk
                for f in carry[:4]:
                    f()
                for ii in range(4):
                    apply_piece(ii)()
                    wo_chunk(12 + ii, (scr_pool, aux_pool)[ii % 2], tail=True)()

    nc.compile()
    names = dict(
        xT=xT_d.name,
        cT=cT_d.name,
        wq=wq_d.name,
        wk=wk_d.name,
        wv=wv_d.name,
        wo=wo_d.name,
        out=out_d.name,
    )
    return nc, names


def _get_built():
    if "nc" not in _CACHE:
        _CACHE["nc"], _CACHE["names"] = _build()
    return _CACHE["nc"], _CACHE["names"]


def run(x, context, Wq, Wk, Wv, Wo, bo, trace=False):
    from concourse.bass_utils import run_bass_kernel_spmd

    nc, names = _get_built()
    bf16 = ml_dtypes.bfloat16

    x = np.asarray(x, dtype=np.float32)
    context = np.asarray(context, dtype=np.float32)
    Wq = np.asarray(Wq, dtype=np.float32)
    Wk = np.asarray(Wk, dtype=np.float32)
    Wv = np.asarray(Wv, dtype=np.float32)
    Wo = np.asarray(Wo, dtype=np.float32)
    bo = np.asarray(bo, dtype=np.float32)

    in_maps = []
    for c in range(8):
        b, g = divmod(c, 2)
        cols = slice(g * HC, (g + 1) * HC)
        in_maps.append(
            {
                names["xT"]: np.ascontiguousarray(x[b].T).astype(bf16),
                names["cT"]: np.ascontiguousarray(context[b].T).astype(bf16),
                names["wq"]: np.ascontiguousarray(Wq[:, cols]).astype(bf16),
                names["wk"]: np.ascontiguousarray(Wk[:, cols]).astype(bf16),
                names["wv"]: np.ascontiguousarray(Wv[:, cols]).astype(bf16),
                names["wo"]: np.ascontiguousarray(Wo[cols, :]).astype(bf16),
            }
        )

    res = run_bass_kernel_spmd(
        nc, in_maps, core_ids=list(range(8)), trace=trace,
        stitch_traces=trace,
    )
    out = np.empty((B, N, DIM), dtype=np.float32)
    for b in range(B):
        out[b] = res.results[2 * b][names["out"]] + res.results[2 * b + 1][names["out"]]
    out += bo[None, None, :]
    return out, res


def kernel(x, context, Wq, Wk, Wv, Wo, bo):
    out, _ = run(x, context, Wq, Wk, Wv, Wo, bo, trace=False)
    return out
